# revision 16
# baseline (speedup 1.0000x reference)
"""GAT cell (gnn_message_passing) Bass kernel for 8 Trainium2 NeuronCores.

Sharding: pure data parallelism over batch (64 graphs -> 8 per core), both
branches (in/out) on every core.

Math per graph/branch, all layouts chosen so no device transposes are needed:
    x^T  = Wh^T @ input^T                      [att, j]   (pair free-packed)
    s^T  = x @ (x*a)^T  (lhsT=x^T, rhs=xa^T)   [j, i]
    es   = exp(prelu(s^T))                     (scalar ACT, one table set)
    D    = (I+B) @ B     (B=A^T; lhsT=(I+A) natural, rhs=A^T natural; fp8,
                          exact: 0/1 inputs, integer counts in f32 PSUM)
    Mk   = (I+B) @ D     support(Mk) == support(B+2B^2+B^3) == 3-hop mask
    P^T  = es * (Mk>0)   (one fused vector stt; a phantom eps row keeps
                          every rowsum structurally positive)
    Y    = input @ We;  ys = Y + bias  (bias folded: U = P@[Y+bias|1]
                          gives U[:,att] = rowsum and U/rowsum = out + bias)
    U    = P @ [ys|1]    [i, att+1]
    out  = U[:, :att] * (1/U[:, att])   (vector recip + broadcast mult)

Only deviation from the reference: rows whose 3-hop mask is entirely empty
output 0 instead of bias (P(row) ~ 3.5e-5; bounded ~1e-4 relative error).

Host packs (per core): ADJ [128, 8, 912] fp8e4 ((I+A) row-chunked, 2x256
padded cols + A^T row-chunked 2x200), XT [128, 8, 512] bf16 (input^T
h-chunked, j padded to 256), weights replicated. Output [128,8,2,64] bf16.
"""

import numpy as np
from contextlib import ExitStack

import concourse.bass as bass
import concourse.bacc as bacc
import concourse.tile as tile
from concourse import mybir, bass_utils

F32, BF16, FP8 = mybir.dt.float32, mybir.dt.bfloat16, mybir.dt.float8e4
AF = mybir.ActivationFunctionType
ALU = mybir.AluOpType

NCORES = 8
B = 64
BPC = B // NCORES        # batches per core
N = 200                  # nodes per graph
H = 256                  # feature dim
ATT = 64                 # head dim
EPS = 1e-6               # phantom-neighbor weight: rowsum >= EPS always
BRS = ("in", "out")


def _emit(ctx, tc, order, ADJ, XT, WH, WE, AV, BV, O):
    nc = tc.nc
    consts = ctx.enter_context(tc.tile_pool(name="consts", bufs=1))
    pin = ctx.enter_context(tc.tile_pool(name="pin", bufs=1))
    pw = ctx.enter_context(tc.tile_pool(name="pw", bufs=2))
    # PSUM pools (8 banks x 2KB): xt 2 + sc 2 + dm 2 + yo 1 + u 1 = 8
    pxt = ctx.enter_context(tc.tile_pool(name="pxt", bufs=2, space="PSUM"))
    psc = ctx.enter_context(tc.tile_pool(name="psc", bufs=1, space="PSUM"))
    pdm = ctx.enter_context(tc.tile_pool(name="pdm", bufs=2, space="PSUM"))
    pyo = ctx.enter_context(tc.tile_pool(name="pyo", bufs=1, space="PSUM"))
    pu = ctx.enter_context(tc.tile_pool(name="pu", bufs=1, space="PSUM"))

    # ---- PE warmup: fire dummy matmuls from ~2.5us so the HAM clock
    # gate is warm (2.4 GHz) before the real matmul stream begins ----
    scratch = consts.tile([128, 64], BF16, tag="warm", name="warm")
    nc.vector.memset(scratch, 0.0)
    wps = pxt.tile([64, 2, 256], F32, tag="xt_ps", name="warm_ps")
    for _ in range(72):
        nc.tensor.matmul(wps[:, 0, 0:64], scratch, scratch,
                         start=True, stop=True)

    # ---- input tiles first: the 4:8 chunk's DMAs go at the head of the
    # gpsimd queue so the transfer starts immediately ----
    adjt, xtt = {}, {}
    for br in BRS:
        adjt[br] = pin.tile([128, BPC, 912], FP8, tag=f"adj_{br}",
                            name=f"adj_{br}")
        xtt[br] = pin.tile([128, BPC, 512], BF16, tag=f"xt_{br}",
                           name=f"xt_{br}")
    for br in BRS:
        nc.gpsimd.dma_start(out=adjt[br][:, 4:8, :], in_=ADJ[br][:, 4:8, :])
        nc.gpsimd.dma_start(out=xtt[br][:, 4:8, :], in_=XT[br][:, 4:8, :])

    # ---- constants (replicated weights) ----
    wh, we, av, bias = {}, {}, {}, {}
    for br in BRS:
        wh[br] = consts.tile([128, 2, ATT], BF16, tag=f"wh_{br}", name=f"wh_{br}")
        nc.gpsimd.dma_start(out=wh[br], in_=WH[br])
        we[br] = consts.tile([128, 2, ATT], BF16, tag=f"we_{br}", name=f"we_{br}")
        nc.gpsimd.dma_start(out=we[br], in_=WE[br])
        av[br] = consts.tile([64, 1], F32, tag=f"av_{br}", name=f"av_{br}")
        nc.gpsimd.dma_start(out=av[br], in_=AV[br].rearrange("(a o) -> a o", o=1))
        # bias broadcast to [128, 2, ATT] (varies along the free att dim only)
        bias[br] = consts.tile([128, 2, ATT], F32, tag=f"bias_{br}",
                               name=f"bias_{br}")
        bcast = bass.AP(tensor=BV[br].tensor, offset=BV[br].offset,
                        ap=[[0, 128], [0, 2], [1, ATT]])
        nc.gpsimd.dma_start(out=bias[br], in_=bcast)

    # ---- remaining input chunks (pair-aligned) on the sync queue ----
    for lo, hi in [(0, 2), (2, 4)]:
        for br in BRS:
            nc.sync.dma_start(out=adjt[br][:, lo:hi, :],
                              in_=ADJ[br][:, lo:hi, :])
            nc.sync.dma_start(out=xtt[br][:, lo:hi, :],
                              in_=XT[br][:, lo:hi, :])

    def adj_ap(br, b):
        return adjt[br][:, b, :]

    def xt_ap(br, b):
        return xtt[br][:, b, :]

    def xt_pair(br, b0, hc):
        # [128, 2, 256] rhs spanning the pair's hc block
        return xtt[br][:, b0:b0 + 2, hc * 256:(hc + 1) * 256]

    def adj_chunks(br, b):
        a0 = adj_ap(br, b)[:, 0:512].rearrange("p (c m) -> p c m", c=2)
        Tt = adj_ap(br, b)[:, 512:912].rearrange("p (c m) -> p c m", c=2)
        return a0, Tt

    for q in range(4):               # pairs: batches 2q, 2q+1
        b0 = 2 * q
        st = {}                      # per-branch state across phases
        # ---- phase A: x^T matmuls + D = (I+B)@B (no cross-engine deps) ----
        for br in BRS:
            xt_ps = pxt.tile([64, 2, 256], F32, tag="xt_ps", name="xt_ps")
            for hc in range(2):
                nc.tensor.matmul(xt_ps[:, :, :], wh[br][:, hc, :],
                                 xt_pair(br, b0, hc),
                                 start=(hc == 0), stop=(hc == 1))
            xts = pw.tile([64, 2, 256], BF16, tag="xts", name="xts")
            nc.scalar.activation(out=xts, in_=xt_ps, func=AF.Copy)
            xas = pw.tile([64, 2, 256], BF16, tag="xas", name="xas")
            nc.scalar.activation(out=xas, in_=xt_ps, func=AF.Copy,
                                 scale=av[br])
            dsbs = []
            for i in range(2):
                a0, Tt = adj_chunks(br, b0 + i)
                d_ps = pdm.tile([128, 2, 256], F32, tag="dm", name="d_ps")
                for mc in range(2):
                    for kc in range(2):
                        nc.tensor.matmul(
                            d_ps[:, mc, 0:N],
                            a0[:, kc, mc * 128:(mc + 1) * 128],
                            Tt[:, kc, :], start=(kc == 0), stop=(kc == 1))
                dsb = pw.tile([128, 2, 256], FP8, tag="dsb", name="dsb",
                              bufs=4)
                nc.vector.tensor_copy(out=dsb[:, :, 0:N], in_=d_ps[:, :, 0:N])
                dsbs.append(dsb)
            st[br] = {"xts": xts, "xas": xas, "dsbs": dsbs}

        # ---- phase B: scores + Mk = (I+B)@D + P^T + Y ----
        for br in BRS:
            s = st[br]
            sc_ps = psc.tile([128, 2, 2, 256], F32, tag="sc_ps", name="sc_ps")
            for i in range(2):
                for jc in range(2):
                    nc.tensor.matmul(sc_ps[:, i, jc, 0:N],
                                     s["xts"][:, i, jc * 128:(jc + 1) * 128],
                                     s["xas"][:, i, 0:N],
                                     start=True, stop=True)
            nc.scalar.activation(out=sc_ps[:, :, :, 0:N],
                                 in_=sc_ps[:, :, :, 0:N],
                                 func=AF.Prelu, alpha=0.2)
            es = pw.tile([128, 2, 2, N], BF16, tag="es", name="es")
            nc.scalar.activation(out=es, in_=sc_ps[:, :, :, 0:N], func=AF.Exp)
            yo_ps = pyo.tile([128, 2, 2, ATT], F32, tag="yo_ps", name="yo_ps")
            pts = []
            for i in range(2):
                b = b0 + i
                a0, Tt = adj_chunks(br, b)
                m_ps = pdm.tile([128, 2, 256], F32, tag="dm", name="m_ps")
                for mc in range(2):
                    for kc in range(2):
                        nc.tensor.matmul(
                            m_ps[:, mc, 0:N],
                            a0[:, kc, mc * 128:(mc + 1) * 128],
                            s["dsbs"][i][:, kc, 0:N],
                            start=(kc == 0), stop=(kc == 1))
                pt = pw.tile([128, 2, 256], BF16, tag="pt", name="pt", bufs=4)
                if q == 0:
                    # pad cols are never written afterwards; zero once/buf
                    nc.gpsimd.memset(pt[:, :, N:256], 0.0)
                nc.vector.scalar_tensor_tensor(
                    out=pt[:, :, 0:N], in0=m_ps[:, :, 0:N], scalar=0.0,
                    in1=es[:, i, :, :], op0=ALU.is_gt, op1=ALU.mult)
                nc.gpsimd.memset(pt[96:128, 1, 0:N], EPS)
                pts.append(pt)
                for jc in range(2):
                    for hc in range(2):
                        nc.tensor.matmul(
                            yo_ps[:, i, jc, 0:ATT],
                            xt_ap(br, b)[:, hc * 256 + jc * 128:
                                         hc * 256 + (jc + 1) * 128],
                            we[br][:, hc, :], start=(hc == 0), stop=(hc == 1))
            s["yo_ps"] = yo_ps
            s["pts"] = pts

        # ---- phase C: ys, U = P @ [ys|1], normalize, store ----
        for br in BRS:
            s = st[br]
            ys = pw.tile([128, 2, 2, ATT + 1], BF16, tag="ys", name="ys")
            if q < 2:
                # ones column persists across generations (2 bufs)
                nc.gpsimd.memset(ys[:, :, :, ATT:ATT + 1], 1.0)
            bias_bc = bass.AP(
                tensor=bias[br].tensor, offset=bias[br].offset,
                ap=[[bias[br].ap[0][0], 128], [0, 2],
                    [bias[br].ap[1][0], 2], [bias[br].ap[2][0], ATT]])
            nc.vector.scalar_tensor_tensor(
                out=ys[:, :, :, 0:ATT], in0=s["yo_ps"], scalar=1.0,
                in1=bias_bc, op0=ALU.mult, op1=ALU.add)
            u_ps = pu.tile([128, 2, 2, 128], F32, tag="u_ps", name="u_ps")
            for i in range(2):
                for ic in range(2):
                    for jc in range(2):
                        nc.tensor.matmul(
                            u_ps[:, i, ic, 0:ATT + 1],
                            s["pts"][i][:, jc, ic * 128:(ic + 1) * 128],
                            ys[:, i, jc, :], start=(jc == 0), stop=(jc == 1))
            r = pw.tile([128, 2, 2, 1], F32, tag="r", name="r")
            nc.vector.reciprocal(out=r, in_=u_ps[:, :, :, ATT:ATT + 1])
            r_bc = bass.AP(tensor=r.tensor, offset=r.offset,
                           ap=[[r.ap[0][0], 128], [r.ap[1][0], 2],
                               [r.ap[2][0], 2], [0, ATT]])
            rs = pw.tile([128, 2, 2, ATT], BF16, tag="res", name="rs", bufs=3)
            nc.vector.tensor_tensor(out=rs, in0=u_ps[:, :, :, 0:ATT],
                                    in1=r_bc, op=ALU.mult)
            nc.sync.dma_start(out=O[br][:, b0:b0 + 2, :, :], in_=rs)


def _maybe_enable_ldw_opt():
    # The framework passes --enable-ldw-opt=false to walrus; allow opting
    # back in (validated by our own rel-err check) via env toggle.
    import os
    if os.environ.get("KERNEL_LDW_OPT") != "1":
        return
    if getattr(bass_utils, "_ldw_opt_patched", False):
        return
    orig = bass_utils.run_command

    def patched(cmd, *a, **kw):
        if isinstance(cmd, list):
            cmd = ["--enable-ldw-opt=true" if c == "--enable-ldw-opt=false"
                   else c for c in cmd]
        return orig(cmd, *a, **kw)

    bass_utils.run_command = patched
    bass_utils._ldw_opt_patched = True


def build(order: int) -> bacc.Bacc:
    assert order == 3, "kernel compiled for order=3 reachability"
    _maybe_enable_ldw_opt()
    import os
    nd = 1 if os.environ.get("KERNEL_ND1") == "1" else NCORES
    asserts = os.environ.get("KERNEL_ASSERTS", "0") == "1"
    nc = bacc.Bacc("TRN2", target_bir_lowering=False, debug=False,
                   enable_asserts=asserts, num_devices=nd)
    ADJ, XT, WH, WE, AV, BV, O = {}, {}, {}, {}, {}, {}, {}
    for br in BRS:
        ADJ[br] = nc.dram_tensor(f"ADJ_{br}", [128, BPC, 912], FP8,
                                 kind="ExternalInput").ap()
        XT[br] = nc.dram_tensor(f"XT_{br}", [128, BPC, 512], BF16,
                                kind="ExternalInput").ap()
        WH[br] = nc.dram_tensor(f"WH_{br}", [128, 2, ATT], BF16,
                                kind="ExternalInput").ap()
        WE[br] = nc.dram_tensor(f"WE_{br}", [128, 2, ATT], BF16,
                                kind="ExternalInput").ap()
        AV[br] = nc.dram_tensor(f"AV_{br}", [64], F32,
                                kind="ExternalInput").ap()
        BV[br] = nc.dram_tensor(f"BV_{br}", [ATT], F32,
                                kind="ExternalInput").ap()
        O[br] = nc.dram_tensor(f"O_{br}", [128, BPC, 2, ATT], BF16,
                               kind="ExternalOutput").ap()
    with tile.TileContext(nc) as tc:
        with ExitStack() as ctx:
            _emit(ctx, tc, order, ADJ, XT, WH, WE, AV, BV, O)
    nc.compile()
    return nc


_CACHE = {}


def _get(order: int) -> bacc.Bacc:
    if order not in _CACHE:
        _CACHE[order] = build(order)
    return _CACHE[order]


def _dtypes():
    import ml_dtypes
    return ml_dtypes.bfloat16, ml_dtypes.float8_e4m3fn


def make_in_maps(A_in_0, A_out_0, input_in, input_out,
                 W_head_in, W_head_out, a_in, a_out,
                 W_edge_in, W_edge_out, bias_iah, bias_oah):
    bf, f8 = _dtypes()
    per = {
        "in": (A_in_0, input_in, W_head_in, W_edge_in, a_in, bias_iah),
        "out": (A_out_0, input_out, W_head_out, W_edge_out, a_out, bias_oah),
    }
    shared = {}
    packed = {}
    eye = np.eye(N, dtype=np.float32)
    for br, (A, X, Wh, We, a, bv) in per.items():
        A = np.asarray(A, np.float32)
        X = np.asarray(X, np.float32)
        bsz = A.shape[0]
        # ADJ [b, 128, 912] fp8: (I+A) row-chunked (2x256 pad) + A^T (2x200)
        adj = np.zeros((bsz, 128, 912), dtype=f8)
        aI = (A + eye[None]).astype(f8)
        adj[:, 0:128, 0:200] = aI[:, 0:128, :]
        adj[:, 0:72, 256:456] = aI[:, 128:200, :]
        At = np.ascontiguousarray(np.transpose(A, (0, 2, 1))).astype(f8)
        adj[:, 0:128, 512:712] = At[:, 0:128, :]
        adj[:, 0:72, 712:912] = At[:, 128:200, :]
        # XT [b, 128, 512] bf16: input^T h-chunked, j padded 200->256
        xt = np.zeros((bsz, 128, 512), dtype=bf)
        Xt = np.transpose(X, (0, 2, 1)).astype(bf)   # [b, 256, 200]
        xt[:, :, 0:200] = Xt[:, 0:128, :]
        xt[:, :, 256:456] = Xt[:, 128:256, :]
        packed[br] = (np.ascontiguousarray(np.transpose(adj, (1, 0, 2))),
                      np.ascontiguousarray(np.transpose(xt, (1, 0, 2))))
        Whb = np.asarray(Wh, np.float32).astype(bf)
        shared[f"WH_{br}"] = np.ascontiguousarray(
            np.stack([Whb[0:128], Whb[128:256]], axis=1))
        Web = np.asarray(We, np.float32).astype(bf)
        shared[f"WE_{br}"] = np.ascontiguousarray(
            np.stack([Web[0:128], Web[128:256]], axis=1))
        shared[f"AV_{br}"] = np.ascontiguousarray(a, dtype=np.float32)
        shared[f"BV_{br}"] = np.ascontiguousarray(bv, dtype=np.float32)
    shards = []
    for c in range(NCORES):
        s = slice(c * BPC, (c + 1) * BPC)
        m = dict(shared)
        for br in BRS:
            adj_pm, xt_pm = packed[br]
            m[f"ADJ_{br}"] = np.ascontiguousarray(adj_pm[:, s, :])
            m[f"XT_{br}"] = np.ascontiguousarray(xt_pm[:, s, :])
        shards.append(m)
    return shards


def run(trace=False, **inputs):
    order = int(inputs.get("order", 3))
    nc = _get(order)
    in_maps = make_in_maps(
        A_in_0=inputs["A_in_0"], A_out_0=inputs["A_out_0"],
        input_in=inputs["input_in"], input_out=inputs["input_out"],
        W_head_in=inputs["W_head_in"], W_head_out=inputs["W_head_out"],
        a_in=inputs["a_in"], a_out=inputs["a_out"],
        W_edge_in=inputs["W_edge_in"], W_edge_out=inputs["W_edge_out"],
        bias_iah=inputs["bias_iah"], bias_oah=inputs["bias_oah"])
    kw2 = {}
    if trace:
        import os
        td = os.path.join(os.getcwd(), "trace_out")
        os.makedirs(td, exist_ok=True)
        kw2["tmpdir"] = td
    res = bass_utils.run_bass_kernel_spmd(nc, in_maps,
                                          core_ids=list(range(NCORES)),
                                          trace=trace, **kw2)
    outs = {}
    for br in BRS:
        parts = []
        for c in range(NCORES):
            o = np.asarray(res.results[c][f"O_{br}"])    # [128, 8, 2, 64]
            o = np.transpose(o.astype(np.float32), (1, 2, 0, 3))
            parts.append(o.reshape(BPC, 256, ATT)[:, 0:N, :])
        outs[br] = np.concatenate(parts, axis=0)
    return (outs["in"], outs["out"]), res


def kernel(**inputs):
    (out_in, out_out), _ = run(trace=False, **inputs)
    return out_in, out_out


# revision 17
# speedup vs baseline: 1.1247x; 1.1247x over previous
"""GAT cell (gnn_message_passing) Bass kernel for 8 Trainium2 NeuronCores.

Sharding: pure data parallelism over batch (64 graphs -> 8 per core), both
branches (in/out) on every core.

Math per graph/branch, all layouts chosen so no device transposes are needed:
    x^T  = Wh^T @ input^T                      [att, j]   (pair free-packed)
    s^T  = x @ (x*a)^T  (lhsT=x^T, rhs=xa^T)   [j, i]
    es   = exp(prelu(s^T))                     (scalar ACT, one table set)
    D    = (I+B) @ B     (B=A^T; lhsT=(I+A) natural, rhs=A^T natural; fp8,
                          exact: 0/1 inputs, integer counts in f32 PSUM)
    Mk   = (I+B) @ D     support(Mk) == support(B+2B^2+B^3) == 3-hop mask
    P^T  = es * (Mk>0)   (one fused vector stt; a phantom eps row keeps
                          every rowsum structurally positive)
    Y    = input @ We;  ys = Y + bias  (bias folded: U = P@[Y+bias|1]
                          gives U[:,att] = rowsum and U/rowsum = out + bias)
    U    = P @ [ys|1]    [i, att+1]
    out  = U[:, :att] * (1/U[:, att])   (vector recip + broadcast mult)

Only deviation from the reference: rows whose 3-hop mask is entirely empty
output 0 instead of bias (P(row) ~ 3.5e-5; bounded ~1e-4 relative error).

Host packs (per core): ADJ [128, 8, 912] fp8e4 ((I+A) row-chunked, 2x256
padded cols + A^T row-chunked 2x200), XT [128, 8, 512] bf16 (input^T
h-chunked, j padded to 256), weights replicated. Output [128,8,2,64] bf16.
"""

import numpy as np
from contextlib import ExitStack

import concourse.bass as bass
import concourse.bacc as bacc
import concourse.tile as tile
from concourse import mybir, bass_utils

F32, BF16, FP8 = mybir.dt.float32, mybir.dt.bfloat16, mybir.dt.float8e4
AF = mybir.ActivationFunctionType
ALU = mybir.AluOpType

NCORES = 8
B = 64
BPC = B // NCORES        # batches per core
N = 200                  # nodes per graph
H = 256                  # feature dim
ATT = 64                 # head dim
EPS = 1e-6               # phantom-neighbor weight: rowsum >= EPS always
BRS = ("in", "out")


def _emit(ctx, tc, order, ADJ, XT, WH, WE, AV, BV, O):
    nc = tc.nc
    consts = ctx.enter_context(tc.tile_pool(name="consts", bufs=1))
    pin = ctx.enter_context(tc.tile_pool(name="pin", bufs=1))
    pw = ctx.enter_context(tc.tile_pool(name="pw", bufs=2))
    # PSUM pools (8 banks x 2KB): xt 2 + sc 2 + dm 2 + yo 1 + u 1 = 8
    pxt = ctx.enter_context(tc.tile_pool(name="pxt", bufs=2, space="PSUM"))
    psc = ctx.enter_context(tc.tile_pool(name="psc", bufs=1, space="PSUM"))
    pdm = ctx.enter_context(tc.tile_pool(name="pdm", bufs=2, space="PSUM"))
    pyo = ctx.enter_context(tc.tile_pool(name="pyo", bufs=1, space="PSUM"))
    pu = ctx.enter_context(tc.tile_pool(name="pu", bufs=1, space="PSUM"))

    # ---- PE warmup: fire dummy matmuls from ~2.5us so the HAM clock
    # gate is warm (2.4 GHz) before the real matmul stream begins ----
    scratch = consts.tile([128, 64], BF16, tag="warm", name="warm")
    nc.vector.memset(scratch, 0.0)
    wps = pxt.tile([64, 2, 256], F32, tag="xt_ps", name="warm_ps")
    for _ in range(72):
        nc.tensor.matmul(wps[:, 0, 0:64], scratch, scratch,
                         start=True, stop=True)

    # ---- constants (replicated weights) ----
    wh, we, av, bias = {}, {}, {}, {}
    for br in BRS:
        wh[br] = consts.tile([128, 2, ATT], BF16, tag=f"wh_{br}", name=f"wh_{br}")
        nc.gpsimd.dma_start(out=wh[br], in_=WH[br])
        we[br] = consts.tile([128, 2, ATT], BF16, tag=f"we_{br}", name=f"we_{br}")
        nc.gpsimd.dma_start(out=we[br], in_=WE[br])
        av[br] = consts.tile([64, 1], F32, tag=f"av_{br}", name=f"av_{br}")
        nc.gpsimd.dma_start(out=av[br], in_=AV[br].rearrange("(a o) -> a o", o=1))
        # bias broadcast to [128, 2, ATT] (varies along the free att dim only)
        bias[br] = consts.tile([128, 2, ATT], F32, tag=f"bias_{br}",
                               name=f"bias_{br}")
        bcast = bass.AP(tensor=BV[br].tensor, offset=BV[br].offset,
                        ap=[[0, 128], [0, 2], [1, ATT]])
        nc.gpsimd.dma_start(out=bias[br], in_=bcast)

    # ---- input loads: chunks of 2/2/4 batches (pair-aligned); the early
    # chunks go on sync (issued first, transfer uncontested), the big
    # 4:8 chunk on gpsimd behind the small const DMAs ----
    adjt, xtt = {}, {}
    for br in BRS:
        adjt[br] = pin.tile([128, BPC, 912], FP8, tag=f"adj_{br}",
                            name=f"adj_{br}")
        xtt[br] = pin.tile([128, BPC, 512], BF16, tag=f"xt_{br}",
                           name=f"xt_{br}")
    for ci, (lo, hi) in enumerate([(0, 2), (2, 4), (4, 8)]):
        for br in BRS:
            eng = (nc.sync, nc.sync, nc.gpsimd)[ci]
            eng.dma_start(out=adjt[br][:, lo:hi, :], in_=ADJ[br][:, lo:hi, :])
            eng.dma_start(out=xtt[br][:, lo:hi, :], in_=XT[br][:, lo:hi, :])

    def adj_ap(br, b):
        return adjt[br][:, b, :]

    def xt_ap(br, b):
        return xtt[br][:, b, :]

    def xt_pair(br, b0, hc):
        # [128, 2, 256] rhs spanning the pair's hc block
        return xtt[br][:, b0:b0 + 2, hc * 256:(hc + 1) * 256]

    def adj_chunks(br, b):
        a0 = adj_ap(br, b)[:, 0:512].rearrange("p (c m) -> p c m", c=2)
        Tt = adj_ap(br, b)[:, 512:912].rearrange("p (c m) -> p c m", c=2)
        return a0, Tt

    for q in range(4):               # pairs: batches 2q, 2q+1
        b0 = 2 * q
        st = {}                      # per-branch state across phases
        # ---- phase A: x^T matmuls + D = (I+B)@B (no cross-engine deps) ----
        for br in BRS:
            xt_ps = pxt.tile([64, 2, 256], F32, tag="xt_ps", name="xt_ps")
            for hc in range(2):
                nc.tensor.matmul(xt_ps[:, :, :], wh[br][:, hc, :],
                                 xt_pair(br, b0, hc),
                                 start=(hc == 0), stop=(hc == 1))
            xts = pw.tile([64, 2, 256], BF16, tag="xts", name="xts")
            nc.scalar.activation(out=xts, in_=xt_ps, func=AF.Copy)
            xas = pw.tile([64, 2, 256], BF16, tag="xas", name="xas")
            nc.scalar.activation(out=xas, in_=xt_ps, func=AF.Copy,
                                 scale=av[br])
            dsbs = []
            for i in range(2):
                a0, Tt = adj_chunks(br, b0 + i)
                d_ps = pdm.tile([128, 2, 256], F32, tag="dm", name="d_ps")
                for mc in range(2):
                    for kc in range(2):
                        nc.tensor.matmul(
                            d_ps[:, mc, 0:N],
                            a0[:, kc, mc * 128:(mc + 1) * 128],
                            Tt[:, kc, :], start=(kc == 0), stop=(kc == 1))
                dsb = pw.tile([128, 2, 256], FP8, tag="dsb", name="dsb",
                              bufs=4)
                nc.vector.tensor_copy(out=dsb[:, :, 0:N], in_=d_ps[:, :, 0:N])
                dsbs.append(dsb)
            st[br] = {"xts": xts, "xas": xas, "dsbs": dsbs}

        # ---- phase B: scores + Mk = (I+B)@D + P^T + Y ----
        for br in BRS:
            s = st[br]
            sc_ps = psc.tile([128, 2, 2, 256], F32, tag="sc_ps", name="sc_ps")
            for i in range(2):
                for jc in range(2):
                    nc.tensor.matmul(sc_ps[:, i, jc, 0:N],
                                     s["xts"][:, i, jc * 128:(jc + 1) * 128],
                                     s["xas"][:, i, 0:N],
                                     start=True, stop=True)
            nc.scalar.activation(out=sc_ps[:, :, :, 0:N],
                                 in_=sc_ps[:, :, :, 0:N],
                                 func=AF.Prelu, alpha=0.2)
            es = pw.tile([128, 2, 2, N], BF16, tag="es", name="es")
            nc.scalar.activation(out=es, in_=sc_ps[:, :, :, 0:N], func=AF.Exp)
            yo_ps = pyo.tile([128, 2, 2, ATT], F32, tag="yo_ps", name="yo_ps")
            pts = []
            for i in range(2):
                b = b0 + i
                a0, Tt = adj_chunks(br, b)
                m_ps = pdm.tile([128, 2, 256], F32, tag="dm", name="m_ps")
                for mc in range(2):
                    for kc in range(2):
                        nc.tensor.matmul(
                            m_ps[:, mc, 0:N],
                            a0[:, kc, mc * 128:(mc + 1) * 128],
                            s["dsbs"][i][:, kc, 0:N],
                            start=(kc == 0), stop=(kc == 1))
                pt = pw.tile([128, 2, 256], BF16, tag="pt", name="pt", bufs=4)
                if q == 0:
                    # pad cols are never written afterwards; zero once/buf
                    nc.gpsimd.memset(pt[:, :, N:256], 0.0)
                nc.vector.scalar_tensor_tensor(
                    out=pt[:, :, 0:N], in0=m_ps[:, :, 0:N], scalar=0.0,
                    in1=es[:, i, :, :], op0=ALU.is_gt, op1=ALU.mult)
                nc.gpsimd.memset(pt[96:128, 1, 0:N], EPS)
                pts.append(pt)
                for jc in range(2):
                    for hc in range(2):
                        nc.tensor.matmul(
                            yo_ps[:, i, jc, 0:ATT],
                            xt_ap(br, b)[:, hc * 256 + jc * 128:
                                         hc * 256 + (jc + 1) * 128],
                            we[br][:, hc, :], start=(hc == 0), stop=(hc == 1))
            s["yo_ps"] = yo_ps
            s["pts"] = pts

        # ---- phase C: ys, U = P @ [ys|1], normalize, store ----
        for br in BRS:
            s = st[br]
            ys = pw.tile([128, 2, 2, ATT + 1], BF16, tag="ys", name="ys")
            if q < 2:
                # ones column persists across generations (2 bufs)
                nc.gpsimd.memset(ys[:, :, :, ATT:ATT + 1], 1.0)
            bias_bc = bass.AP(
                tensor=bias[br].tensor, offset=bias[br].offset,
                ap=[[bias[br].ap[0][0], 128], [0, 2],
                    [bias[br].ap[1][0], 2], [bias[br].ap[2][0], ATT]])
            nc.vector.scalar_tensor_tensor(
                out=ys[:, :, :, 0:ATT], in0=s["yo_ps"], scalar=1.0,
                in1=bias_bc, op0=ALU.mult, op1=ALU.add)
            u_ps = pu.tile([128, 2, 2, 128], F32, tag="u_ps", name="u_ps")
            for i in range(2):
                for ic in range(2):
                    for jc in range(2):
                        nc.tensor.matmul(
                            u_ps[:, i, ic, 0:ATT + 1],
                            s["pts"][i][:, jc, ic * 128:(ic + 1) * 128],
                            ys[:, i, jc, :], start=(jc == 0), stop=(jc == 1))
            r = pw.tile([128, 2, 2, 1], F32, tag="r", name="r")
            nc.vector.reciprocal(out=r, in_=u_ps[:, :, :, ATT:ATT + 1])
            r_bc = bass.AP(tensor=r.tensor, offset=r.offset,
                           ap=[[r.ap[0][0], 128], [r.ap[1][0], 2],
                               [r.ap[2][0], 2], [0, ATT]])
            rs = pw.tile([128, 2, 2, ATT], BF16, tag="res", name="rs", bufs=3)
            nc.vector.tensor_tensor(out=rs, in0=u_ps[:, :, :, 0:ATT],
                                    in1=r_bc, op=ALU.mult)
            nc.sync.dma_start(out=O[br][:, b0:b0 + 2, :, :], in_=rs)


def _maybe_enable_ldw_opt():
    # The framework passes --enable-ldw-opt=false to walrus; allow opting
    # back in (validated by our own rel-err check) via env toggle.
    import os
    if os.environ.get("KERNEL_LDW_OPT") != "1":
        return
    if getattr(bass_utils, "_ldw_opt_patched", False):
        return
    orig = bass_utils.run_command

    def patched(cmd, *a, **kw):
        if isinstance(cmd, list):
            cmd = ["--enable-ldw-opt=true" if c == "--enable-ldw-opt=false"
                   else c for c in cmd]
        return orig(cmd, *a, **kw)

    bass_utils.run_command = patched
    bass_utils._ldw_opt_patched = True


def build(order: int) -> bacc.Bacc:
    assert order == 3, "kernel compiled for order=3 reachability"
    _maybe_enable_ldw_opt()
    import os
    nd = 1 if os.environ.get("KERNEL_ND1") == "1" else NCORES
    asserts = os.environ.get("KERNEL_ASSERTS", "0") == "1"
    nc = bacc.Bacc("TRN2", target_bir_lowering=False, debug=False,
                   enable_asserts=asserts, num_devices=nd)
    ADJ, XT, WH, WE, AV, BV, O = {}, {}, {}, {}, {}, {}, {}
    for br in BRS:
        ADJ[br] = nc.dram_tensor(f"ADJ_{br}", [128, BPC, 912], FP8,
                                 kind="ExternalInput").ap()
        XT[br] = nc.dram_tensor(f"XT_{br}", [128, BPC, 512], BF16,
                                kind="ExternalInput").ap()
        WH[br] = nc.dram_tensor(f"WH_{br}", [128, 2, ATT], BF16,
                                kind="ExternalInput").ap()
        WE[br] = nc.dram_tensor(f"WE_{br}", [128, 2, ATT], BF16,
                                kind="ExternalInput").ap()
        AV[br] = nc.dram_tensor(f"AV_{br}", [64], F32,
                                kind="ExternalInput").ap()
        BV[br] = nc.dram_tensor(f"BV_{br}", [ATT], F32,
                                kind="ExternalInput").ap()
        O[br] = nc.dram_tensor(f"O_{br}", [128, BPC, 2, ATT], BF16,
                               kind="ExternalOutput").ap()
    with tile.TileContext(nc) as tc:
        with ExitStack() as ctx:
            _emit(ctx, tc, order, ADJ, XT, WH, WE, AV, BV, O)
    nc.compile()
    return nc


_CACHE = {}


def _get(order: int) -> bacc.Bacc:
    if order not in _CACHE:
        _CACHE[order] = build(order)
    return _CACHE[order]


def _dtypes():
    import ml_dtypes
    return ml_dtypes.bfloat16, ml_dtypes.float8_e4m3fn


def make_in_maps(A_in_0, A_out_0, input_in, input_out,
                 W_head_in, W_head_out, a_in, a_out,
                 W_edge_in, W_edge_out, bias_iah, bias_oah):
    bf, f8 = _dtypes()
    per = {
        "in": (A_in_0, input_in, W_head_in, W_edge_in, a_in, bias_iah),
        "out": (A_out_0, input_out, W_head_out, W_edge_out, a_out, bias_oah),
    }
    shared = {}
    packed = {}
    eye = np.eye(N, dtype=np.float32)
    for br, (A, X, Wh, We, a, bv) in per.items():
        A = np.asarray(A, np.float32)
        X = np.asarray(X, np.float32)
        bsz = A.shape[0]
        # ADJ [b, 128, 912] fp8: (I+A) row-chunked (2x256 pad) + A^T (2x200)
        adj = np.zeros((bsz, 128, 912), dtype=f8)
        aI = (A + eye[None]).astype(f8)
        adj[:, 0:128, 0:200] = aI[:, 0:128, :]
        adj[:, 0:72, 256:456] = aI[:, 128:200, :]
        At = np.ascontiguousarray(np.transpose(A, (0, 2, 1))).astype(f8)
        adj[:, 0:128, 512:712] = At[:, 0:128, :]
        adj[:, 0:72, 712:912] = At[:, 128:200, :]
        # XT [b, 128, 512] bf16: input^T h-chunked, j padded 200->256
        xt = np.zeros((bsz, 128, 512), dtype=bf)
        Xt = np.transpose(X, (0, 2, 1)).astype(bf)   # [b, 256, 200]
        xt[:, :, 0:200] = Xt[:, 0:128, :]
        xt[:, :, 256:456] = Xt[:, 128:256, :]
        packed[br] = (np.ascontiguousarray(np.transpose(adj, (1, 0, 2))),
                      np.ascontiguousarray(np.transpose(xt, (1, 0, 2))))
        Whb = np.asarray(Wh, np.float32).astype(bf)
        shared[f"WH_{br}"] = np.ascontiguousarray(
            np.stack([Whb[0:128], Whb[128:256]], axis=1))
        Web = np.asarray(We, np.float32).astype(bf)
        shared[f"WE_{br}"] = np.ascontiguousarray(
            np.stack([Web[0:128], Web[128:256]], axis=1))
        shared[f"AV_{br}"] = np.ascontiguousarray(a, dtype=np.float32)
        shared[f"BV_{br}"] = np.ascontiguousarray(bv, dtype=np.float32)
    shards = []
    for c in range(NCORES):
        s = slice(c * BPC, (c + 1) * BPC)
        m = dict(shared)
        for br in BRS:
            adj_pm, xt_pm = packed[br]
            m[f"ADJ_{br}"] = np.ascontiguousarray(adj_pm[:, s, :])
            m[f"XT_{br}"] = np.ascontiguousarray(xt_pm[:, s, :])
        shards.append(m)
    return shards


def run(trace=False, **inputs):
    order = int(inputs.get("order", 3))
    nc = _get(order)
    in_maps = make_in_maps(
        A_in_0=inputs["A_in_0"], A_out_0=inputs["A_out_0"],
        input_in=inputs["input_in"], input_out=inputs["input_out"],
        W_head_in=inputs["W_head_in"], W_head_out=inputs["W_head_out"],
        a_in=inputs["a_in"], a_out=inputs["a_out"],
        W_edge_in=inputs["W_edge_in"], W_edge_out=inputs["W_edge_out"],
        bias_iah=inputs["bias_iah"], bias_oah=inputs["bias_oah"])
    kw2 = {}
    if trace:
        import os
        td = os.path.join(os.getcwd(), "trace_out")
        os.makedirs(td, exist_ok=True)
        kw2["tmpdir"] = td
    res = bass_utils.run_bass_kernel_spmd(nc, in_maps,
                                          core_ids=list(range(NCORES)),
                                          trace=trace, **kw2)
    outs = {}
    for br in BRS:
        parts = []
        for c in range(NCORES):
            o = np.asarray(res.results[c][f"O_{br}"])    # [128, 8, 2, 64]
            o = np.transpose(o.astype(np.float32), (1, 2, 0, 3))
            parts.append(o.reshape(BPC, 256, ATT)[:, 0:N, :])
        outs[br] = np.concatenate(parts, axis=0)
    return (outs["in"], outs["out"]), res


def kernel(**inputs):
    (out_in, out_out), _ = run(trace=False, **inputs)
    return out_in, out_out
